# revision 3
# baseline (speedup 1.0000x reference)
"""KV-cache scatter update kernel for Trainium2 (8 NeuronCores), in-place.

Problem: kv_cache (2L=4, B=8, H=8, S=4096, D=128) f32, new_kv (L=2, 2, B=8,
H=8, 1, D=128) f32, position_ids (B=8, 1) int. Output = kv_cache with
new_kv[l, kv, b, h, 0, :] written at [2l+kv, b, h, pos[b], :].

Strategy: the output is the input cache with only 32 rows-of-128 updated per
core, so the kernel must not re-copy the 512 MB cache. The Bass program's
ExternalOutput is aliased onto the kv input buffer via bass_jit's
lowering_input_output_aliases (target_bir_lowering/NKI path) and the kv
device buffer is donated, so the NEFF performs ONLY the scatter: stage the
32 new rows + 32 precomputed row indices in SBUF, then one indirect DMA
writes them into the cache at the runtime offsets. HBM traffic per core
drops from 134 MB (bulk copy) to ~50 KB.

Sharding: the (2L*B = 32) contiguous [H, S, D] blocks are split 4-per-core
across the 8 cores, so every host-side slice is a zero-copy reshape view of
the full arrays (the concatenation of the per-core shards IS the natural
layout of kv_cache and new_kv). Scatter row index for per-core row
p = 8*j + h (block j, head h) is p*S + pos[b(j)], computed on host.
"""

import numpy as np

L = 2
B = 8
H = 8
S = 4096
D = 128
NCORES = 8
LP = 2 * L               # 4 cache planes (k/v interleaved)
BLOCKS = LP * B          # 32 [H, S, D] blocks, 4 per core
ROWS = (BLOCKS // NCORES) * H * S   # 131072 rows of D f32 per core
NEW = (BLOCKS // NCORES) * H        # 32 scattered rows per core

_STATE = None


def _build():
    """Build the jitted SPMD executor (one NEFF, same program on 8 cores)."""
    global _STATE
    if _STATE is not None:
        return _STATE

    import jax
    from jax.sharding import Mesh, NamedSharding, PartitionSpec

    from jax.experimental.shard_map import shard_map

    import concourse.bass as bass
    import concourse.mybir as mybir
    import concourse.tile as tile
    from concourse.bass2jax import bass_jit

    @bass_jit(
        target_bir_lowering=True,
        lowering_input_output_aliases={0: 0},
        trn_type="TRN2",
        num_devices=NCORES,
    )
    def _scatter(nc, kv, newkv, idx):
        # `out` aliases the kv input buffer — no copy, scatter-only.
        out = nc.dram_tensor("out", [ROWS, D], mybir.dt.float32, kind="ExternalOutput")
        with tile.TileContext(nc) as tc:
            with tc.tile_pool(name="sb", bufs=1) as pool:
                newt = pool.tile([NEW, D], mybir.dt.float32)
                idxt = pool.tile([NEW, 1], mybir.dt.int32)
                nc.gpsimd.dma_start(out=newt[:], in_=newkv[:])
                nc.gpsimd.dma_start(out=idxt[:], in_=idx[:])
                nc.gpsimd.indirect_dma_start(
                    out=out[:],
                    out_offset=bass.IndirectOffsetOnAxis(ap=idxt[:, :1], axis=0),
                    in_=newt[:],
                    in_offset=None,
                )
        return (out,)

    mesh = Mesh(np.asarray(jax.devices()[:NCORES]), ("core",))
    P = PartitionSpec
    fn = jax.jit(
        shard_map(
            _scatter,
            mesh=mesh,
            in_specs=(P("core"), P("core"), P("core")),
            out_specs=P("core"),
            check_rep=False,
        ),
        donate_argnums=(0,),
    )
    sharding = NamedSharding(mesh, P("core"))
    _STATE = (jax, fn, sharding)
    return _STATE


def kernel(kv_cache, new_kv, position_ids):
    jax, fn, sharding = _build()

    kv_cache = np.asarray(kv_cache)
    new_kv = np.asarray(new_kv)
    position_ids = np.asarray(position_ids)

    # Global inputs in per-core-concatenated layout — zero-copy views.
    kv_g = kv_cache.reshape(NCORES * ROWS, D)
    new_g = new_kv.reshape(NCORES * NEW, D)
    # global row r = 32*core + p, p = 8*j + h; block g = 4*core + j has
    # batch b = g % B and scatter row index p*S + pos[b] within the core.
    r = np.arange(NCORES * NEW)
    idx_g = ((r % NEW) * S + position_ids[(r // H) % B, 0]).astype(np.int32)
    idx_g = np.ascontiguousarray(idx_g.reshape(NCORES * NEW, 1))

    kv_d = jax.device_put(kv_g, sharding)
    new_d = jax.device_put(new_g, sharding)
    idx_d = jax.device_put(idx_g, sharding)
    (out_d,) = fn(kv_d, new_d, idx_d)
    return np.asarray(out_d).reshape(LP, B, H, S, D)


# revision 4
# speedup vs baseline: 1.1270x; 1.1270x over previous
"""KV-cache scatter update kernel for Trainium2 (8 NeuronCores), in-place.

Problem: kv_cache (2L=4, B=8, H=8, S=4096, D=128) f32, new_kv (L=2, 2, B=8,
H=8, 1, D=128) f32, position_ids (B=8, 1) int. Output = kv_cache with
new_kv[l, kv, b, h, 0, :] written at [2l+kv, b, h, pos[b], :].

Strategy: the output is the input cache with only 32 rows-of-128 updated per
core, so the kernel must not re-copy the 512 MB cache. The Bass program's
ExternalOutput is aliased onto the kv input buffer via bass_jit's
lowering_input_output_aliases (target_bir_lowering/NKI path) and the kv
device buffer is donated, so the NEFF performs ONLY the scatter: stage the
32 new rows + 32 precomputed row indices in SBUF, then one indirect DMA
writes them into the cache at the runtime offsets. HBM traffic per core
drops from 134 MB (bulk copy) to ~50 KB.

Sharding: the (2L*B = 32) contiguous [H, S, D] blocks are split 4-per-core
across the 8 cores, so every host-side slice is a zero-copy reshape view of
the full arrays (the concatenation of the per-core shards IS the natural
layout of kv_cache and new_kv). Scatter row index for per-core row
p = 8*j + h (block j, head h) is p*S + pos[b(j)], computed on host.
"""

import numpy as np

L = 2
B = 8
H = 8
S = 4096
D = 128
NCORES = 8
LP = 2 * L               # 4 cache planes (k/v interleaved)
BLOCKS = LP * B          # 32 [H, S, D] blocks, 4 per core
ROWS = (BLOCKS // NCORES) * H * S   # 131072 rows of D f32 per core
NEW = (BLOCKS // NCORES) * H        # 32 scattered rows per core

_STATE = None


def _build():
    """Build the jitted SPMD executor (one NEFF, same program on 8 cores)."""
    global _STATE
    if _STATE is not None:
        return _STATE

    import jax
    from jax.sharding import Mesh, NamedSharding, PartitionSpec

    from jax.experimental.shard_map import shard_map

    import concourse.bass as bass
    import concourse.mybir as mybir
    import concourse.tile as tile
    from concourse.bass2jax import bass_jit

    @bass_jit(
        target_bir_lowering=True,
        lowering_input_output_aliases={0: 0},
        trn_type="TRN2",
        num_devices=NCORES,
    )
    def _scatter(nc, kv, newkv, idx):
        # `out` aliases the kv input buffer — no copy, scatter-only.
        out = nc.dram_tensor("out", [ROWS, D], mybir.dt.float32, kind="ExternalOutput")
        with tile.TileContext(nc) as tc:
            with tc.tile_pool(name="sb", bufs=1) as pool:
                newt = pool.tile([NEW, D], mybir.dt.float32)
                idxt = pool.tile([NEW, 1], mybir.dt.int32)
                # HWDGE loads (sync/scalar rings) run in parallel and leave
                # only the indirect scatter on the SWDGE/gpsimd path.
                nc.sync.dma_start(out=newt[:], in_=newkv[:])
                nc.scalar.dma_start(out=idxt[:], in_=idx[:])
                nc.gpsimd.indirect_dma_start(
                    out=out[:],
                    out_offset=bass.IndirectOffsetOnAxis(ap=idxt[:, :1], axis=0),
                    in_=newt[:],
                    in_offset=None,
                )
        return (out,)

    mesh = Mesh(np.asarray(jax.devices()[:NCORES]), ("core",))
    P = PartitionSpec
    fn = jax.jit(
        shard_map(
            _scatter,
            mesh=mesh,
            in_specs=(P("core"), P("core"), P("core")),
            out_specs=P("core"),
            check_rep=False,
        ),
        donate_argnums=(0,),
    )
    sharding = NamedSharding(mesh, P("core"))
    _STATE = (jax, fn, sharding)
    return _STATE


def kernel(kv_cache, new_kv, position_ids):
    jax, fn, sharding = _build()

    kv_cache = np.asarray(kv_cache)
    new_kv = np.asarray(new_kv)
    position_ids = np.asarray(position_ids)

    # Global inputs in per-core-concatenated layout — zero-copy views.
    kv_g = kv_cache.reshape(NCORES * ROWS, D)
    new_g = new_kv.reshape(NCORES * NEW, D)
    # global row r = 32*core + p, p = 8*j + h; block g = 4*core + j has
    # batch b = g % B and scatter row index p*S + pos[b] within the core.
    r = np.arange(NCORES * NEW)
    idx_g = ((r % NEW) * S + position_ids[(r // H) % B, 0]).astype(np.int32)
    idx_g = np.ascontiguousarray(idx_g.reshape(NCORES * NEW, 1))

    kv_d = jax.device_put(kv_g, sharding)
    new_d = jax.device_put(new_g, sharding)
    idx_d = jax.device_put(idx_g, sharding)
    (out_d,) = fn(kv_d, new_d, idx_d)
    return np.asarray(out_d).reshape(LP, B, H, S, D)


# revision 7
# speedup vs baseline: 1.7504x; 1.5532x over previous
"""KV-cache scatter update kernel for Trainium2 (8 NeuronCores), in-place.

Problem: kv_cache (2L=4, B=8, H=8, S=4096, D=128) f32, new_kv (L=2, 2, B=8,
H=8, 1, D=128) f32, position_ids (B=8, 1) int. Output = kv_cache with
new_kv[l, kv, b, h, 0, :] written at [2l+kv, b, h, pos[b], :].

Strategy: the output is the input cache with only 32 rows-of-128 updated per
core, so the kernel must not re-copy the 512 MB cache. The Bass program's
ExternalOutput is aliased onto the kv input buffer via bass_jit's
lowering_input_output_aliases (target_bir_lowering/NKI path) and the kv
device buffer is donated, so the NEFF performs ONLY the scatter. The device
job is two chained DMAs — the latency floor for a dependent scatter:
  1. one HWDGE load of a host-packed [32, 129] f32 tile (128 payload cols +
     the int32 row index bitcast into col 128) into SBUF;
  2. one SWDGE indirect DMA that writes the 32 rows into the cache at the
     runtime offsets.
HBM traffic per core drops from 134 MB (bulk copy) to ~50 KB, and per-job
device time from ~380 us to ~4.5 us (2 x ~2 us DMA completion receipts).

Sharding: the (2L*B = 32) contiguous [H, S, D] blocks are split 4-per-core
across the 8 cores, so every host-side slice is a zero-copy reshape view of
the full arrays (the concatenation of the per-core shards IS the natural
layout of kv_cache and new_kv). Scatter row index for per-core row
p = 8*j + h (block j, head h) is p*S + pos[b(j)], computed on host.
"""

import numpy as np

L = 2
B = 8
H = 8
S = 4096
D = 128
NCORES = 8
LP = 2 * L               # 4 cache planes (k/v interleaved)
BLOCKS = LP * B          # 32 [H, S, D] blocks, 4 per core
ROWS = (BLOCKS // NCORES) * H * S   # 131072 rows of D f32 per core
NEW = (BLOCKS // NCORES) * H        # 32 scattered rows per core

_STATE = None


def _build():
    """Build the jitted SPMD executor (one NEFF, same program on 8 cores)."""
    global _STATE
    if _STATE is not None:
        return _STATE

    import jax
    from jax.sharding import Mesh, NamedSharding, PartitionSpec

    from jax.experimental.shard_map import shard_map

    import concourse.bass as bass
    import concourse.mybir as mybir
    import concourse.tile as tile
    from concourse.bass2jax import bass_jit

    @bass_jit(
        target_bir_lowering=True,
        lowering_input_output_aliases={0: 0},
        trn_type="TRN2",
        num_devices=NCORES,
    )
    def _scatter(nc, kv, packed):
        # `out` aliases the kv input buffer — no copy, scatter-only.
        out = nc.dram_tensor("out", [ROWS, D], mybir.dt.float32, kind="ExternalOutput")
        with tile.TileContext(nc) as tc:
            with tc.tile_pool(name="sb", bufs=1) as pool:
                newt = pool.tile([NEW, D + 1], mybir.dt.float32)
                nc.sync.dma_start(out=newt[:], in_=packed[:])
                nc.gpsimd.indirect_dma_start(
                    out=out[:],
                    out_offset=bass.IndirectOffsetOnAxis(
                        ap=newt[:, D : D + 1].bitcast(mybir.dt.int32), axis=0
                    ),
                    in_=newt[:, :D],
                    in_offset=None,
                )
        return (out,)

    mesh = Mesh(np.asarray(jax.devices()[:NCORES]), ("core",))
    P = PartitionSpec
    fn = jax.jit(
        shard_map(
            _scatter,
            mesh=mesh,
            in_specs=(P("core"), P("core")),
            out_specs=P("core"),
            check_rep=False,
        ),
        donate_argnums=(0,),
    )
    sharding = NamedSharding(mesh, P("core"))
    _STATE = (jax, fn, sharding)
    return _STATE


def kernel(kv_cache, new_kv, position_ids):
    jax, fn, sharding = _build()

    kv_cache = np.asarray(kv_cache)
    new_kv = np.asarray(new_kv)
    position_ids = np.asarray(position_ids)

    # Global inputs in per-core-concatenated layout; kv is a zero-copy view.
    kv_g = kv_cache.reshape(NCORES * ROWS, D)
    # global row r = 32*core + p, p = 8*j + h; block g = 4*core + j has
    # batch b = g % B and scatter row index p*S + pos[b] within the core.
    # Pack payload + bitcast row index into one [32, 129] f32 row per entry.
    r = np.arange(NCORES * NEW)
    idx_g = ((r % NEW) * S + position_ids[(r // H) % B, 0]).astype(np.int32)
    packed = np.empty((NCORES * NEW, D + 1), np.float32)
    packed[:, :D] = new_kv.reshape(NCORES * NEW, D)
    packed[:, D] = idx_g.view(np.float32)

    kv_d = jax.device_put(kv_g, sharding)
    pk_d = jax.device_put(packed, sharding)
    (out_d,) = fn(kv_d, pk_d)
    return np.asarray(out_d).reshape(LP, B, H, S, D)
